# revision 3
# baseline (speedup 1.0000x reference)
"""DynamicConv (MoE-routed 1x1 conv) Trainium2 kernel.

Data-parallel over batch: 8 cores x 4 samples. Each core:
  - routing MLP (3-layer, exact GELU) + softmax on its 4 samples
  - mixes the K=8 expert kernels per sample (DVE AXPY chain)
  - per-sample 256x256 @ 256x4096 matmul on TensorE (fp32)

Problem constants are hardcoded (self-contained; no sibling imports):
  x [32, 256, 4096] f32, embedding [32, 128] f32,
  W0 [128,128], b0 [128], W1 [128,128], b1 [128], W2 [128,8], b2 [8],
  weight [8, 256, 256, 1], bias_k [8, 256]  -> out [32, 256, 4096] f32
"""

import numpy as np

import concourse.bass as bass
import concourse.bacc as bacc
import concourse.mybir as mybir
import concourse.tile as tile
from concourse import bass_utils

F32 = mybir.dt.float32
AF = mybir.ActivationFunctionType
ALU = mybir.AluOpType

N_CORES = 8
BS = 32
BPC = BS // N_CORES  # samples per core
IN_C = 256
OUT_C = 256
H = 4096
K = 8
D_EMBD = 128
HID = 128
N_IT = IN_C // 128   # input-channel tiles
N_OT = OUT_C // 128  # output-channel tiles
HC = 512             # h-chunk (one PSUM bank of fp32)
N_HC = H // HC

_PROG = None  # compiled program cache


def _build_program():
    nc = bacc.Bacc("TRN2", target_bir_lowering=False, debug=False)

    xs = nc.dram_tensor("xs", [BPC, IN_C, H], F32, kind="ExternalInput").ap()
    embT = nc.dram_tensor("embT", [D_EMBD, BPC], F32, kind="ExternalInput").ap()
    w0 = nc.dram_tensor("w0", [D_EMBD, HID], F32, kind="ExternalInput").ap()
    b0 = nc.dram_tensor("b0", [HID, 1], F32, kind="ExternalInput").ap()
    w1 = nc.dram_tensor("w1", [HID, HID], F32, kind="ExternalInput").ap()
    b1 = nc.dram_tensor("b1", [HID, 1], F32, kind="ExternalInput").ap()
    w2 = nc.dram_tensor("w2", [HID, K], F32, kind="ExternalInput").ap()
    b2 = nc.dram_tensor("b2", [K, 1], F32, kind="ExternalInput").ap()
    # wt[k, it, il, o] = weight[k, o, it*128+il]  (pre-transposed on host)
    wt = nc.dram_tensor("wt", [K, N_IT, 128, OUT_C], F32, kind="ExternalInput").ap()
    bk = nc.dram_tensor("bk", [K, OUT_C], F32, kind="ExternalInput").ap()
    id8 = nc.dram_tensor("id8", [K, K], F32, kind="ExternalInput").ap()
    out = nc.dram_tensor("out", [BPC, OUT_C, H], F32, kind="ExternalOutput").ap()

    with tile.TileContext(nc) as tc:
        with (
            tc.tile_pool(name="consts", bufs=1) as cpool,
            tc.tile_pool(name="rsb", bufs=1) as rsb,
            tc.tile_pool(name="rps", bufs=2, space="PSUM") as rps,
            tc.tile_pool(name="mix", bufs=4) as mixp,
            tc.tile_pool(name="xin", bufs=2) as xinp,
            tc.tile_pool(name="osb", bufs=2) as osbp,
            tc.tile_pool(name="mps", bufs=6, space="PSUM") as mps,
        ):
            # ---- constant loads ----
            w0_sb = cpool.tile([D_EMBD, HID], F32, tag="w0")
            nc.sync.dma_start(w0_sb[:], w0[:])
            w1_sb = cpool.tile([HID, HID], F32, tag="w1")
            nc.sync.dma_start(w1_sb[:], w1[:])
            w2_sb = cpool.tile([HID, K], F32, tag="w2")
            nc.sync.dma_start(w2_sb[:], w2[:])
            b0_sb = cpool.tile([HID, 1], F32, tag="b0")
            nc.sync.dma_start(b0_sb[:], b0[:])
            b1_sb = cpool.tile([HID, 1], F32, tag="b1")
            nc.sync.dma_start(b1_sb[:], b1[:])
            b2_sb = cpool.tile([K, 1], F32, tag="b2")
            nc.sync.dma_start(b2_sb[:], b2[:])
            bk_sb = cpool.tile([K, OUT_C], F32, tag="bk")
            nc.sync.dma_start(bk_sb[:], bk[:])
            id8_sb = cpool.tile([K, K], F32, tag="id8")
            nc.sync.dma_start(id8_sb[:], id8[:])
            embT_sb = cpool.tile([D_EMBD, BPC], F32, tag="embT")
            nc.sync.dma_start(embT_sb[:], embT[:])
            ones_sb = cpool.tile([1, 128], F32, tag="ones")
            nc.vector.memset(ones_sb[:], 1.0)

            wt_sb = {}
            for k in range(K):
                for it in range(N_IT):
                    t = cpool.tile([128, OUT_C], F32, tag=f"wt{k}_{it}",
                                   name=f"wt{k}_{it}")
                    nc.sync.dma_start(t[:], wt[k, it, :, :])
                    wt_sb[k, it] = t

            # ---- routing MLP (transposed orientation, all 4 samples) ----
            p1 = rps.tile([HID, BPC], F32, tag="rp")
            nc.tensor.matmul(p1[:], w0_sb[:], embT_sb[:], start=True, stop=True)
            h1_sb = rsb.tile([HID, BPC], F32, tag="h1")
            nc.scalar.activation(h1_sb[:], p1[:], AF.Gelu, bias=b0_sb[:, 0:1])

            p2 = rps.tile([HID, BPC], F32, tag="rp")
            nc.tensor.matmul(p2[:], w1_sb[:], h1_sb[:], start=True, stop=True)
            h2_sb = rsb.tile([HID, BPC], F32, tag="h2")
            nc.scalar.activation(h2_sb[:], p2[:], AF.Gelu, bias=b1_sb[:, 0:1])

            p3 = rps.tile([K, BPC], F32, tag="rp")
            nc.tensor.matmul(p3[:], w2_sb[:], h2_sb[:], start=True, stop=True)
            lT_sb = rsb.tile([K, BPC], F32, tag="lT")
            nc.scalar.activation(lT_sb[:], p3[:], AF.Identity, bias=b2_sb[:, 0:1])

            # logitsT [K, BPC] -> logits [BPC, K]; softmax over free dim.
            # Logits are O(1) here so exp without max-subtraction is safe.
            p4 = rps.tile([BPC, K], F32, tag="rp")
            nc.tensor.transpose(p4[:], lT_sb[:], id8_sb[:])
            e_sb = rsb.tile([BPC, K], F32, tag="e")
            nc.scalar.activation(e_sb[:], p4[:], AF.Exp)
            s_sb = rsb.tile([BPC, 1], F32, tag="s")
            nc.vector.reduce_sum(s_sb[:], e_sb[:], axis=mybir.AxisListType.X)
            r_sb = rsb.tile([BPC, 1], F32, tag="r")
            nc.vector.reciprocal(r_sb[:], s_sb[:])
            att_sb = rsb.tile([BPC, K], F32, tag="att")
            nc.vector.tensor_scalar_mul(att_sb[:], e_sb[:], r_sb[:, 0:1])

            # attT [K, BPC] for the bias mix
            p5 = rps.tile([K, BPC], F32, tag="rp")
            nc.tensor.transpose(p5[:], att_sb[:], id8_sb[0:BPC, 0:BPC])
            attT_sb = rsb.tile([K, BPC], F32, tag="attT")
            nc.vector.tensor_copy(attT_sb[:], p5[:])

            # agg_bT[ot] [128, BPC] = bias_k[:, ot].T @ att.T
            aggb_sb = []
            for ot in range(N_OT):
                p6 = rps.tile([128, BPC], F32, tag="rp")
                nc.tensor.matmul(p6[:], bk_sb[:, ot * 128:(ot + 1) * 128],
                                 attT_sb[:], start=True, stop=True)
                a = rsb.tile([128, BPC], F32, tag=f"aggb{ot}", name=f"aggb{ot}")
                nc.vector.tensor_copy(a[:], p6[:])
                aggb_sb.append(a)

            # broadcast att to all 128 partitions: attB [128, BPC*K]
            att_flat = rsb.tile([1, BPC * K], F32, tag="attf")
            nc.sync.dma_start(att_flat[:], att_sb[:])
            p7 = rps.tile([128, BPC * K], F32, tag="rp")
            nc.tensor.matmul(p7[:], ones_sb[:], att_flat[:], start=True, stop=True)
            attB_sb = rsb.tile([128, BPC * K], F32, tag="attB")
            nc.vector.tensor_copy(attB_sb[:], p7[:])

            # ---- mix expert kernels + main per-sample matmul ----
            for b in range(BPC):
                mixT = []
                for it in range(N_IT):
                    m = mixp.tile([128, OUT_C], F32, tag=f"mix{it}",
                                  name=f"mix_b{b}_{it}")
                    a0 = attB_sb[:, b * K:b * K + 1]
                    nc.vector.tensor_scalar_mul(m[:], wt_sb[0, it][:], a0)
                    for k in range(1, K):
                        ak = attB_sb[:, b * K + k:b * K + k + 1]
                        nc.vector.scalar_tensor_tensor(
                            m[:], wt_sb[k, it][:], ak, m[:],
                            op0=ALU.mult, op1=ALU.add)
                    mixT.append(m)

                x_t = []
                for it in range(N_IT):
                    xt = xinp.tile([128, H], F32, tag=f"x{it}",
                                   name=f"x_b{b}_{it}")
                    nc.sync.dma_start(xt[:], xs[b, it * 128:(it + 1) * 128, :])
                    x_t.append(xt)

                for ot in range(N_OT):
                    o_sb = osbp.tile([128, H], F32, tag=f"o{ot}",
                                     name=f"o_b{b}_{ot}")
                    for hc in range(N_HC):
                        ps = mps.tile([128, HC], F32, tag="mm")
                        for it in range(N_IT):
                            nc.tensor.matmul(
                                ps[:],
                                mixT[it][:, ot * 128:(ot + 1) * 128],
                                x_t[it][:, hc * HC:(hc + 1) * HC],
                                start=(it == 0), stop=(it == N_IT - 1))
                        dst = o_sb[:, hc * HC:(hc + 1) * HC]
                        bias_ap = aggb_sb[ot][:, b:b + 1]
                        if hc % 2 == 0:
                            nc.scalar.activation(dst, ps[:], AF.Identity,
                                                 bias=bias_ap)
                        else:
                            nc.vector.tensor_scalar(dst, ps[:], bias_ap, None,
                                                    op0=ALU.add)
                    nc.sync.dma_start(out[b, ot * 128:(ot + 1) * 128, :], o_sb[:])

    nc.compile()
    return nc


def _get_program():
    global _PROG
    if _PROG is None:
        _PROG = _build_program()
    return _PROG


def build_in_maps(inputs):
    x = np.ascontiguousarray(np.asarray(inputs["x"], dtype=np.float32))
    emb = np.asarray(inputs["embedding"], dtype=np.float32)
    W0 = np.ascontiguousarray(np.asarray(inputs["W0"], dtype=np.float32))
    b0 = np.ascontiguousarray(np.asarray(inputs["b0"], dtype=np.float32).reshape(HID, 1))
    W1 = np.ascontiguousarray(np.asarray(inputs["W1"], dtype=np.float32))
    b1 = np.ascontiguousarray(np.asarray(inputs["b1"], dtype=np.float32).reshape(HID, 1))
    W2 = np.ascontiguousarray(np.asarray(inputs["W2"], dtype=np.float32))
    b2 = np.ascontiguousarray(np.asarray(inputs["b2"], dtype=np.float32).reshape(K, 1))
    weight = np.asarray(inputs["weight"], dtype=np.float32)[..., 0]  # [K, O, I]
    bias_k = np.ascontiguousarray(np.asarray(inputs["bias_k"], dtype=np.float32))

    # wt[k, it, il, o] = weight[k, o, it*128+il]
    wt = np.ascontiguousarray(
        weight.transpose(0, 2, 1).reshape(K, N_IT, 128, OUT_C))
    id8 = np.eye(K, dtype=np.float32)

    in_maps = []
    for c in range(N_CORES):
        sl = slice(c * BPC, (c + 1) * BPC)
        in_maps.append({
            "xs": np.ascontiguousarray(x[sl]),
            "embT": np.ascontiguousarray(emb[sl].T),
            "w0": W0, "b0": b0, "w1": W1, "b1": b1, "w2": W2, "b2": b2,
            "wt": wt, "bk": bias_k, "id8": id8,
        })
    return in_maps


def run(inputs, trace=False):
    nc = _get_program()
    in_maps = build_in_maps(inputs)
    res = bass_utils.run_bass_kernel_spmd(
        nc, in_maps, core_ids=list(range(N_CORES)), trace=trace)
    out = np.concatenate([res.results[c]["out"] for c in range(N_CORES)], axis=0)
    return out, res


def kernel(**inputs):
    out, _ = run(inputs, trace=False)
    return out
